# revision 49
# baseline (speedup 1.0000x reference)
"""Trainium2 8-core kernel for a dense pre-norm transformer block.

Reference: h=LN1(x); qkv=h@w_qkv; causal MHA (16 heads, Dh=64);
x+=o@w_out; h2=LN2(x); x+=gelu(h2@w1+b1)@w2+b2.

Sharding (Megatron TP-8 + sequence-parallel residual):
  - heads 2c,2c+1 on core c (w_qkv column-shard, w_out row-shard)
  - MLP hidden 512 per core (w1 column-shard, w2 row-shard)
  - residual stream token-sharded: core c owns the four strided pieces
    {q*1024 + c*128 .. +128}, q=0..3. RS1 runs as four quarter-sized
    ReduceScatters fired as soon as the out-projection of each quarter
    is done (the first two hide under attention of batch 1); the
    AllGather runs per half; RS2 runs as four quarters, the first three
    hidden under remaining MLP work.
  - LN1 stats (mean/rstd) are computed shard-locally with bn_stats and
    exchanged via a tiny AllGather at kernel start; LN gains/biases are
    folded into w_qkv/w1 host-side; LN1 mean-subtraction is folded into
    the qkv matmul as a rank-1 (-colsum(w) x mean) accumulation and the
    rstd scaling is applied to the matmul output.

Compute dtype: bf16 operands, fp32 PSUM accumulation, fp32 residual.
Attention scores are computed transposed ST=[k_pos, q_pos]; both heads
run concurrently in the PE array via tile_position row-packing; softmax
denominator comes from a ones-column appended to V; causality via 4
static [128,512] masks.
"""
import numpy as np

import concourse.bass as bass
import concourse.mybir as mybir
import concourse.tile as tile
from concourse import bacc
from concourse import bass_utils
from concourse.masks import make_identity

F32 = mybir.dt.float32
BF = mybir.dt.bfloat16
AF = mybir.ActivationFunctionType

NCORES = 8
B, L, D = 2, 2048, 1024
T = B * L              # 4096 tokens
TSH = T // NCORES      # 512 tokens per core (4 pieces of 128)
DH = 64                # head dim
HL = 2                 # heads per core
DLOC = HL * DH         # 128 local head features
MLPH = 4096 // NCORES  # 512 local hidden
LN_EPS = 1e-5
NT = T // 512          # 8 token tiles of 512
ND = D // 128          # 8 feature chunks
QT = L // 512          # 4 q-tiles per batch

_CACHE = {}


def build():
    if "nc" in _CACHE:
        return _CACHE["nc"]
    nc = bacc.Bacc("TRN2", target_bir_lowering=False, debug=False,
                   num_devices=NCORES)

    xt_in = nc.dram_tensor("xt", [D, T], BF, kind="ExternalInput")
    xsh_in = nc.dram_tensor("xsh", [TSH, D], F32, kind="ExternalInput")
    wqkv_in = nc.dram_tensor("wqkv", [D, 3 * DLOC], BF, kind="ExternalInput")
    nws_in = nc.dram_tensor("nws", [1, 3 * DLOC], BF, kind="ExternalInput")
    bqkv_in = nc.dram_tensor("bqkv", [3 * DLOC, 1], F32, kind="ExternalInput")
    wout_in = nc.dram_tensor("wout", [DLOC, D], BF, kind="ExternalInput")
    w1_in = nc.dram_tensor("w1", [D, MLPH], BF, kind="ExternalInput")
    b1g_in = nc.dram_tensor("b1g", [MLPH, 1], F32, kind="ExternalInput")
    w2_in = nc.dram_tensor("w2", [MLPH, D], BF, kind="ExternalInput")
    b2b_in = nc.dram_tensor("b2b", [128, D], F32, kind="ExternalInput")
    masks_in = nc.dram_tensor("masks", [4, 128, 512], BF, kind="ExternalInput")
    out_ext = nc.dram_tensor("out", [TSH, D], F32, kind="ExternalOutput")

    rg = [list(range(NCORES))]

    with tile.TileContext(nc) as tc:
        with (
            tc.tile_pool(name="const", bufs=1) as const,
            tc.tile_pool(name="wpool", bufs=1) as wpool,
            tc.tile_pool(name="dram", bufs=1, space="DRAM") as dram,
        ):
            # ---- DRAM scratch for collectives ----
            st_ag_in = dram.tile([8, 128], BF)       # (piece q, mean/rstd)
            st_ag_out = dram.tile([64, 128], BF, addr_space="Shared")
            rs1_in = [dram.tile([1024, D], BF, name=f"rs1_in{q}")
                      for q in range(4)]
            rs1_out = [dram.tile([128, D], BF, name=f"rs1_out{q}")
                       for q in range(4)]
            ag_in = [dram.tile([D, 256], BF, name=f"ag_in{g}")
                     for g in range(2)]
            ag_out = [dram.tile([NCORES * D, 256], BF, addr_space="Shared",
                                name=f"ag_out{g}") for g in range(2)]
            rs2_in = [dram.tile([1024, D], BF, name=f"rs2_in{q}")
                      for q in range(4)]
            rs2_out = [dram.tile([128, D], BF, name=f"rs2_out{q}")
                       for q in range(4)]

            warm_in = dram.tile([8, 16], BF)
            warm_out = dram.tile([64, 16], BF, addr_space="Shared")

            # ---- constants ----
            ident = const.tile([128, 128], F32)
            make_identity(nc, ident[:])
            ident_bf = const.tile([128, 128], BF)
            make_identity(nc, ident_bf[:])
            ones_row = const.tile([1, 128], BF)
            nc.vector.memset(ones_row[:], 1.0)
            eps128 = const.tile([128, 1], F32)
            nc.vector.memset(eps128[:], LN_EPS)
            masks_sb = [const.tile([128, 512], BF, name=f"mask{m}")
                        for m in range(4)]

            def load_masks():
                for m in range(4):
                    nc.sync.dma_start(masks_sb[m][:], masks_in.ap()[m])

            # ---- weights resident in SBUF ----
            wqkv_sb = []
            for d in range(ND):
                wt = wpool.tile([128, 3 * DLOC], BF, name=f"wqkv{d}")
                nc.sync.dma_start(wt[:], wqkv_in.ap()[d * 128:(d + 1) * 128, :])
                wqkv_sb.append(wt)
            nws_sb = wpool.tile([1, 3 * DLOC], BF)
            nc.sync.dma_start(nws_sb[:], nws_in.ap())
            bqkv_sb = []
            for m in range(3):
                bt = wpool.tile([128, 1], F32, name=f"bqkv{m}")
                nc.sync.dma_start(bt[:], bqkv_in.ap()[m * 128:(m + 1) * 128, :])
                bqkv_sb.append(bt)
            wout_sb = wpool.tile([DLOC, D], BF)
            w1_sb = [wpool.tile([128, MLPH], BF, name=f"w1_{d}")
                     for d in range(ND)]
            b1g_sb = [wpool.tile([128, 1], F32, name=f"b1g{m}")
                      for m in range(MLPH // 128)]
            w2_sb = [wpool.tile([128, D], BF, name=f"w2_{m}")
                     for m in range(MLPH // 128)]

            def load_late_weights():
                nc.sync.dma_start(wout_sb[:], wout_in.ap())
                for d in range(ND):
                    nc.sync.dma_start(w1_sb[d][:],
                                      w1_in.ap()[d * 128:(d + 1) * 128, :])
                for m in range(MLPH // 128):
                    nc.sync.dma_start(
                        b1g_sb[m][:], b1g_in.ap()[m * 128:(m + 1) * 128, :])
                for m in range(MLPH // 128):
                    nc.sync.dma_start(w2_sb[m][:],
                                      w2_in.ap()[m * 128:(m + 1) * 128, :])

            # persistent activations
            attn_pool_cm = tc.tile_pool(name="attn", bufs=1)
            attn_pool = attn_pool_cm.__enter__()
            qkvT = []
            for m in range(3):
                t_ = attn_pool.tile([128, T], BF, name=f"qkvT{m}")
                qkvT.append(t_)
            oT = attn_pool.tile([128, T], BF)

            resid_pool_cm = tc.tile_pool(name="resid", bufs=1)
            resid_pool = resid_pool_cm.__enter__()
            xsv = resid_pool.tile([128, 4, D], F32)   # my shard of x
            x2_sb = resid_pool.tile([128, 4, D], F32)

            # ========== stage 0: shard-local LN1 stats + tiny AG ==========
            s0_cm = tc.tile_pool(name="s0", bufs=2)
            s0 = s0_cm.__enter__()
            ps0_cm = tc.tile_pool(name="ps0", bufs=2, space="PSUM")
            ps0 = ps0_cm.__enter__()
            # absorb first-collective init latency with a no-dep dummy
            wtile = s0.tile([8, 16], BF, tag="wtile", name="wtile")
            nc.vector.memset(wtile[:], 0.0)
            nc.scalar.dma_start(warm_in[:], wtile[:])
            nc.gpsimd.collective_compute(
                "AllGather", mybir.AluOpType.bypass, replica_groups=rg,
                ins=[warm_in[:].opt()], outs=[warm_out[:].opt()])
            for q in range(4):
                nc.scalar.dma_start(
                    xsv[:, q, :], xsh_in.ap()[q * 128:(q + 1) * 128, :])
                stats = s0.tile([128, 2, 6], F32, tag="stats", name="stats")
                xv = xsv[:, q, :].rearrange("p (s f) -> p s f", s=2)
                for s in range(2):
                    nc.vector.bn_stats(stats[:, s, :], xv[:, s, :])
                mv = s0.tile([128, 2], F32, tag="mv", name="mv")
                nc.vector.bn_aggr(mv[:], stats[:])
                rstd0 = s0.tile([128, 1], F32, tag="rstd0", name="rstd0")
                nc.scalar.activation(rstd0[:], mv[:, 1:2], AF.Sqrt,
                                     bias=eps128[:])
                nc.vector.reciprocal_approx_fast(rstd0[:], rstd0[:])
                st2 = s0.tile([128, 2], BF, tag="st2", name="st2")
                nc.vector.tensor_copy(st2[:, 0:1], mv[:, 0:1])
                nc.vector.tensor_copy(st2[:, 1:2], rstd0[:])
                stp = ps0.tile([2, 128], BF, tag="stp", name="stp")
                nc.tensor.transpose(stp[:], st2[:], ident_bf[:])
                sts = s0.tile([2, 128], BF, tag="sts", name="sts")
                nc.vector.tensor_copy(sts[:], stp[:])
                nc.scalar.dma_start(st_ag_in[2 * q:2 * q + 2, :], sts[:])
            nc.gpsimd.collective_compute(
                "AllGather", mybir.AluOpType.bypass, replica_groups=rg,
                ins=[st_ag_in[:].opt()], outs=[st_ag_out[:].opt()])
            ps0_cm.__exit__(None, None, None)
            s0_cm.__exit__(None, None, None)

            # st_ag_out rows: c*8 + q*2 + {0:mean, 1:rstd}
            st_view = st_ag_out[:].rearrange("(c x) f -> c x f", x=8)

            # ================= stage 1: LN1 + qkv =================
            s1_x_cm = tc.tile_pool(name="s1_x", bufs=2)
            s1_x = s1_x_cm.__enter__()
            s1_tmp_cm = tc.tile_pool(name="s1_tmp", bufs=4)
            s1_tmp = s1_tmp_cm.__enter__()
            s1_stat_cm = tc.tile_pool(name="s1_stat", bufs=2)
            s1_stat = s1_stat_cm.__enter__()
            ps_qkv_cm = tc.tile_pool(name="ps_qkv", bufs=5, space="PSUM")
            ps_qkv = ps_qkv_cm.__enter__()

            def do_s1(tt):
                q4, h4 = tt // 2, tt % 2
                xts = s1_x.tile([128, ND, 512], BF, tag="xts")
                for d in range(ND):
                    eng = nc.sync if d % 2 == 0 else nc.scalar
                    eng.dma_start(
                        xts[:, d, :],
                        xt_in.ap()[d * 128:(d + 1) * 128,
                                   tt * 512:(tt + 1) * 512])
                mean_bf = s1_stat.tile([1, 4, 128], BF, tag="mean_bf")
                rstd_bf = s1_stat.tile([1, 4, 128], BF, tag="rstd_bf")
                nc.scalar.dma_start(
                    mean_bf[:], st_view[4 * h4:4 * h4 + 4, 2 * q4, :])
                nc.scalar.dma_start(
                    rstd_bf[:], st_view[4 * h4:4 * h4 + 4, 2 * q4 + 1, :])
                mean_v = mean_bf[:].rearrange("p a f -> p (a f)")
                rstd_v = rstd_bf[:].rearrange("p a f -> p (a f)")
                ps_qs = []
                for m in range(3):
                    ps_q = ps_qkv.tile([128, 512], F32, tag="ps_q",
                                       name="ps_q")
                    for d in range(ND):
                        nc.tensor.matmul(
                            ps_q[:], wqkv_sb[d][:, m * 128:(m + 1) * 128],
                            xts[:, d, :], start=(d == 0), stop=False)
                    ps_qs.append(ps_q)
                for m in range(3):
                    nc.tensor.matmul(
                        ps_qs[m][:], nws_sb[:, m * 128:(m + 1) * 128],
                        mean_v, start=False, stop=True)
                rstd_b = ps_qkv.tile([128, 512], F32, tag="ps_q",
                                     name="rstd_b")
                nc.tensor.matmul(rstd_b[:], ones_row[:], rstd_v,
                                 start=True, stop=True)
                rstd_bc = s1_tmp.tile([128, 512], BF, tag="rstd_bc")
                nc.vector.tensor_copy(rstd_bc[:], rstd_b[:])
                for m in range(3):
                    pre = s1_tmp.tile([128, 512], BF, tag="pre", name="pre")
                    nc.vector.tensor_mul(pre[:], ps_qs[m][:], rstd_bc[:])
                    nc.scalar.activation(
                        qkvT[m][:, tt * 512:(tt + 1) * 512], pre[:],
                        AF.Identity, bias=bqkv_sb[m][:])
            for tt in range(NT):
                do_s1(tt)
            for cm in (ps_qkv_cm, s1_stat_cm, s1_tmp_cm, s1_x_cm):
                cm.__exit__(None, None, None)

            # ============ stage 2/3/4 pools ============
            s2_vaug_cm = tc.tile_pool(name="s2_vaug", bufs=1)
            s2_vaug = s2_vaug_cm.__enter__()
            s2_exp_cm = tc.tile_pool(name="s2_exp", bufs=6)
            s2_exp = s2_exp_cm.__enter__()
            s2_misc_cm = tc.tile_pool(name="s2_misc", bufs=4)
            s2_misc = s2_misc_cm.__enter__()
            s3_r1_cm = tc.tile_pool(name="s3_r1", bufs=4)
            s3_r1 = s3_r1_cm.__enter__()
            s4_t_cm = tc.tile_pool(name="s4_t", bufs=3)
            s4_t = s4_t_cm.__enter__()
            ps_st_cm = tc.tile_pool(name="ps_st", bufs=3, space="PSUM")
            ps_st = ps_st_cm.__enter__()
            ps_o_cm = tc.tile_pool(name="ps_o", bufs=2, space="PSUM")
            ps_o = ps_o_cm.__enter__()
            ps_vt_cm = tc.tile_pool(name="ps_vt", bufs=1, space="PSUM")
            ps_vt = ps_vt_cm.__enter__()

            vaug_cur = [None]

            def do_attn(b, js):
                tok0 = b * L
                if js[0] == 0:
                    vaug = s2_vaug.tile([128, 2, L // 128, DH + 1], BF,
                                        tag="vaug", name="vaug")
                    nc.vector.memset(vaug[:, :, :, DH:DH + 1], 1.0)
                    vaug_cur[0] = vaug
                    for hl in range(HL):
                        hrow = hl * DH
                        vT_u = qkvT[2][hrow:hrow + DH, tok0:tok0 + L]
                        for kc in range(L // 128):
                            pv = ps_vt.tile([128, DH], BF, tag="pv",
                                            name="pv")
                            nc.tensor.transpose(
                                pv[:], vT_u[:, kc * 128:(kc + 1) * 128],
                                ident_bf[hrow:hrow + DH, hrow:hrow + DH])
                            nc.vector.tensor_copy(vaug[:, hl, kc, 0:DH],
                                                  pv[:])
                vaug = vaug_cur[0]
                for j in js:
                    nk = 4 * (j + 1)
                    po = [ps_o.tile([DH + 1, 512], F32, tag=f"po{hl}",
                                    name=f"po{hl}") for hl in range(HL)]
                    for kc in range(nk):
                        dm = kc - (nk - 4)
                        col0 = 128 * dm if dm > 0 else 0
                        w = 512 - col0
                        ests = []
                        for hl in range(HL):
                            hrow = hl * DH
                            qsl = qkvT[0][hrow:hrow + DH,
                                          tok0 + j * 512 + col0:
                                          tok0 + (j + 1) * 512]
                            ksl = qkvT[1][hrow:hrow + DH,
                                          tok0 + kc * 128:tok0 + (kc + 1) * 128]
                            pst = ps_st.tile([128, 512], F32, tag="pst",
                                             name="pst")
                            nc.tensor.matmul(pst[:, :w], ksl, qsl,
                                             start=True, stop=True,
                                             tile_position=(hrow, 0))
                            est = s2_exp.tile([128, 512], BF, tag=f"est{hl}",
                                              name=f"est{hl}")
                            nc.scalar.activation(est[:, :w], pst[:, :w],
                                                 AF.Exp, scale=0.125)
                            if dm >= 0:
                                nc.vector.tensor_mul(
                                    est[:, :w], est[:, :w],
                                    masks_sb[dm][:, col0:])
                            ests.append(est)
                        for hl in range(HL):
                            nc.tensor.matmul(po[hl][:, col0:],
                                             vaug[:, hl, kc, :],
                                             ests[hl][:, :w], start=(kc == 0),
                                             stop=(kc == nk - 1))
                    for hl in range(HL):
                        hrow = hl * DH
                        den = s2_misc.tile([1, 512], F32, tag="den",
                                           name="den")
                        nc.vector.tensor_copy(den[:], po[hl][DH:DH + 1, :])
                        rec1 = s2_misc.tile([1, 512], F32, tag="rec1",
                                            name="rec1")
                        nc.vector.reciprocal_approx_fast(rec1[:], den[:])
                        rec1b = s2_misc.tile([1, 512], BF, tag="rec1b",
                                             name="rec1b")
                        nc.scalar.copy(rec1b[:], rec1[:])
                        rec_b = ps_vt.tile([64, 512], F32, tag="pv",
                                           name="rec_b")
                        nc.tensor.matmul(rec_b[:], ones_row[0:1, 0:64],
                                         rec1b[:], start=True, stop=True)
                        rec_sb = s2_misc.tile([64, 512], BF, tag="rec_sb",
                                              name="rec_sb")
                        nc.vector.tensor_copy(rec_sb[:], rec_b[:])
                        nc.vector.tensor_mul(
                            oT[hrow:hrow + DH,
                               tok0 + j * 512:tok0 + (j + 1) * 512],
                            po[hl][0:DH, :], rec_sb[:])

            def do_oproj(q):
                """out-projection for quarter q (tokens q*1024..+1024) + RS."""
                for tch in range(8):
                    row0 = q * 1024 + tch * 128
                    r1 = s3_r1.tile([128, D], BF, tag="r1", name="r1")
                    for n in range(2):
                        pop = ps_st.tile([128, 512], F32, tag="pst",
                                         name="pop")
                        nc.tensor.matmul(pop[:], oT[:, row0:row0 + 128],
                                         wout_sb[:, n * 512:(n + 1) * 512],
                                         start=True, stop=True)
                        nc.scalar.copy(r1[:, n * 512:(n + 1) * 512], pop[:])
                    nc.sync.dma_start(
                        rs1_in[q][tch * 128:(tch + 1) * 128, :], r1[:])
                nc.gpsimd.collective_compute(
                    "ReduceScatter", mybir.AluOpType.add, replica_groups=rg,
                    ins=[rs1_in[q][:].opt()], outs=[rs1_out[q][:].opt()])

            def do_s4(q):
                """residual + LN2 + transpose for my piece of quarter q."""
                r1s = s4_t.tile([128, D], BF, tag="r1s", name="r1s")
                nc.sync.dma_start(r1s[:], rs1_out[q][:])
                nc.vector.tensor_add(x2_sb[:, q, :], xsv[:, q, :], r1s[:])
                stats = s4_t.tile([128, 2, 6], F32, tag="stats", name="stats")
                x2v = x2_sb[:, q, :].rearrange("p (s f) -> p s f", s=2)
                for s in range(2):
                    nc.vector.bn_stats(stats[:, s, :], x2v[:, s, :])
                mv = s4_t.tile([128, 2], F32, tag="mv", name="mv")
                nc.vector.bn_aggr(mv[:], stats[:])
                rstd2 = s4_t.tile([128, 1], F32, tag="rstd2", name="rstd2")
                nc.scalar.activation(rstd2[:], mv[:, 1:2], AF.Sqrt,
                                     bias=eps128[:])
                nc.vector.reciprocal_approx_fast(rstd2[:], rstd2[:])
                h2 = s4_t.tile([128, D], F32, tag="h2", name="h2")
                nc.vector.tensor_scalar(
                    out=h2[:], in0=x2_sb[:, q, :], scalar1=mv[:, 0:1],
                    scalar2=rstd2[:], op0=mybir.AluOpType.subtract,
                    op1=mybir.AluOpType.mult)
                g, par = q // 2, q % 2
                for d in range(ND):
                    pt = ps_vt.tile([128, 128], F32, tag="pv", name="pt")
                    nc.tensor.transpose(
                        pt[:], h2[:, d * 128:(d + 1) * 128], ident[:])
                    h2t = s4_t.tile([128, 128], BF, tag="h2t", name="h2t")
                    nc.vector.tensor_copy(h2t[:], pt[:])
                    nc.sync.dma_start(
                        ag_in[g][d * 128:(d + 1) * 128,
                                 par * 128:(par + 1) * 128], h2t[:])

            # ---------------- pipelined schedule (front) ----------------
            load_masks()
            load_late_weights()
            do_attn(0, (0, 1))
            do_attn(0, (2, 3))
            do_oproj(0)
            do_oproj(1)
            do_attn(1, (0, 1))   # hides RS1_0 + RS1_1
            do_oproj(2)
            do_s4(0)
            do_s4(1)
            nc.gpsimd.collective_compute(
                "AllGather", mybir.AluOpType.bypass, replica_groups=rg,
                ins=[ag_in[0][:].opt()], outs=[ag_out[0][:].opt()])
            do_attn(1, (2, 3))
            do_oproj(3)
            do_s4(2)
            do_s4(3)
            nc.gpsimd.collective_compute(
                "AllGather", mybir.AluOpType.bypass, replica_groups=rg,
                ins=[ag_in[1][:].opt()], outs=[ag_out[1][:].opt()])

            for cm in (ps_vt_cm, ps_o_cm, ps_st_cm, s4_t_cm,
                       s3_r1_cm, s2_misc_cm, s2_exp_cm, s2_vaug_cm):
                cm.__exit__(None, None, None)

            # ---- stages 5+6 per half: MLP1+GELU, MLP2, RS2 quarters ----
            s5_h_cm = tc.tile_pool(name="s5_h", bufs=3)
            s5_h = s5_h_cm.__enter__()
            s5_g_cm = tc.tile_pool(name="s5_g", bufs=4)
            s5_g = s5_g_cm.__enter__()
            s5_r2_cm = tc.tile_pool(name="s5_r2", bufs=4)
            s5_r2 = s5_r2_cm.__enter__()
            ps_m1_cm = tc.tile_pool(name="ps_m1", bufs=4, space="PSUM")
            ps_m1 = ps_m1_cm.__enter__()
            ps_m2_cm = tc.tile_pool(name="ps_m2", bufs=4, space="PSUM")
            ps_m2 = ps_m2_cm.__enter__()

            def do_mlp2(g, g1s, rp, par):
                """MLP2 for paired blocks (2rp, 2rp+1), quarter-parity par."""
                for blk in range(2):
                    r = 2 * rp + blk
                    cc = blk * 2 + par      # g1 column chunk
                    r2 = s5_r2.tile([128, D], BF, tag="r2", name="r2")
                    for n2 in range(2):
                        pm2 = ps_m2.tile([128, 512], F32, tag="pm2",
                                         name="pm2")
                        for m in range(MLPH // 128):
                            nc.tensor.matmul(
                                pm2[:], g1s[rp][:, m, cc * 128:(cc + 1) * 128],
                                w2_sb[m][:, n2 * 512:(n2 + 1) * 512],
                                start=(m == 0), stop=(m == MLPH // 128 - 1))
                        nc.scalar.copy(r2[:, n2 * 512:(n2 + 1) * 512], pm2[:])
                    nc.sync.dma_start(rs2_in[2 * g + par][r * 128:(r + 1) * 128,
                                                          :], r2[:])

            def do_mlp(g):
                g1s = []
                for rp in range(4):
                    h2ts = s5_h.tile([128, ND, 512], BF, tag="h2ts",
                                     name="h2ts")
                    for d in range(ND):
                        for blk in range(2):
                            r = 2 * rp + blk
                            nc.sync.dma_start(
                                h2ts[:, d, blk * 256:(blk + 1) * 256],
                                ag_out[g][r * D + d * 128:
                                          r * D + (d + 1) * 128, :])
                    g1 = s5_g.tile([128, MLPH // 128, 512], BF, tag="g1",
                                   name="g1")
                    for m in range(MLPH // 128):
                        pm1 = ps_m1.tile([128, 512], F32, tag="pm1",
                                         name="pm1")
                        for d in range(ND):
                            nc.tensor.matmul(
                                pm1[:], w1_sb[d][:, m * 128:(m + 1) * 128],
                                h2ts[:, d, :], start=(d == 0),
                                stop=(d == ND - 1))
                        nc.scalar.activation(g1[:, m, :], pm1[:], AF.Gelu,
                                             bias=b1g_sb[m][:])
                    g1s.append(g1)
                    do_mlp2(g, g1s, rp, 0)
                nc.gpsimd.collective_compute(
                    "ReduceScatter", mybir.AluOpType.add, replica_groups=rg,
                    ins=[rs2_in[2 * g][:].opt()],
                    outs=[rs2_out[2 * g][:].opt()])
                for rp in range(4):
                    do_mlp2(g, g1s, rp, 1)
                nc.gpsimd.collective_compute(
                    "ReduceScatter", mybir.AluOpType.add, replica_groups=rg,
                    ins=[rs2_in[2 * g + 1][:].opt()],
                    outs=[rs2_out[2 * g + 1][:].opt()])

            s7_cm = tc.tile_pool(name="s7", bufs=3)
            s7 = s7_cm.__enter__()
            b2b_sb = s7.tile([128, D], F32, tag="b2b", name="b2b")
            nc.sync.dma_start(b2b_sb[:], b2b_in.ap())

            def do_s7(q):
                r2s = s7.tile([128, D], BF, tag="r2s", name="r2s")
                nc.sync.dma_start(r2s[:], rs2_out[q][:])
                ot = s7.tile([128, D], F32, tag="ot", name="ot")
                nc.vector.tensor_add(ot[:], x2_sb[:, q, :], r2s[:])
                nc.vector.tensor_add(ot[:], ot[:], b2b_sb[:])
                nc.sync.dma_start(
                    out_ext.ap()[q * 128:(q + 1) * 128, :], ot[:])

            do_mlp(0)           # hides RS1_2, RS1_3, AG_1; fires RS2_0, RS2_1
            do_mlp(1)           # hides RS2_0, RS2_1; fires RS2_2, RS2_3
            do_s7(0)
            do_s7(1)
            do_s7(2)
            do_s7(3)

            for cm in (s7_cm, ps_m2_cm, ps_m1_cm, s5_r2_cm, s5_g_cm, s5_h_cm,
                       resid_pool_cm, attn_pool_cm):
                cm.__exit__(None, None, None)

    nc.compile()
    _CACHE["nc"] = nc
    return nc


def shard_rows(c):
    """Global token rows owned by core c (four strided pieces of 128)."""
    return np.concatenate(
        [np.arange(q * 1024 + c * 128, q * 1024 + (c + 1) * 128)
         for q in range(4)])


def make_in_maps(x, ln1_g, ln1_b, w_qkv, w_out, ln2_g, ln2_b, w1, b1, w2, b2):
    import ml_dtypes
    bf16 = ml_dtypes.bfloat16
    x = np.asarray(x, np.float32)
    xf = np.ascontiguousarray(x.reshape(T, D))
    xt = np.ascontiguousarray(xf.T.astype(bf16))
    w_qkv_eff = np.asarray(w_qkv) * np.asarray(ln1_g)[:, None]
    bias_qkv = np.asarray(ln1_b) @ np.asarray(w_qkv)
    w1_eff = np.asarray(w1) * np.asarray(ln2_g)[:, None]
    bias_h1 = np.asarray(ln2_b) @ np.asarray(w1) + np.asarray(b1)
    b2b = np.tile(np.asarray(b2, np.float32)[None, :], (128, 1))
    km = np.arange(128)[:, None]
    qm = np.arange(512)[None, :]
    masks = np.stack([(km + 128 * m <= qm).astype(bf16)
                      for m in range(4)])
    in_maps = []
    for c in range(NCORES):
        cs = slice(c * DLOC, (c + 1) * DLOC)
        wq = np.concatenate(
            [w_qkv_eff[:, cs], w_qkv_eff[:, D:][:, cs],
             w_qkv_eff[:, 2 * D:][:, cs]], axis=1)
        bq = np.concatenate(
            [bias_qkv[cs], bias_qkv[D:][cs], bias_qkv[2 * D:][cs]])
        rows = shard_rows(c)
        in_maps.append({
            "xt": xt,
            "xsh": np.ascontiguousarray(xf[rows]),
            "wqkv": np.ascontiguousarray(wq.astype(bf16)),
            "nws": np.ascontiguousarray(
                (-wq.sum(axis=0)).astype(bf16)).reshape(1, -1),
            "bqkv": np.ascontiguousarray(bq, np.float32).reshape(-1, 1),
            "wout": np.ascontiguousarray(
                np.asarray(w_out)[cs].astype(bf16)),
            "w1": np.ascontiguousarray(
                w1_eff[:, c * MLPH:(c + 1) * MLPH].astype(bf16)),
            "b1g": np.ascontiguousarray(
                bias_h1[c * MLPH:(c + 1) * MLPH], np.float32).reshape(-1, 1),
            "w2": np.ascontiguousarray(
                np.asarray(w2)[c * MLPH:(c + 1) * MLPH].astype(bf16)),
            "b2b": b2b,
            "masks": masks,
        })
    return in_maps


def kernel(**inputs):
    nc = build()
    in_maps = make_in_maps(**inputs)
    res = bass_utils.run_bass_kernel_spmd(
        nc, in_maps, core_ids=list(range(NCORES)))
    out = np.empty((T, D), np.float32)
    for c in range(NCORES):
        out[shard_rows(c)] = res.results[c]["out"]
    return out.reshape(B, L, D).astype(np.float32)
